# revision 6
# baseline (speedup 1.0000x reference)
"""Trainium2 Bass kernel for nn_LBLResNetBiLm.

Computation (see reference): pad sequence with learned boundary vectors,
take 9-tap left/right weighted window sums over the sequence, then run 3
residual pre-LN FFN layers per branch; emit every layer's concat(left,right).

Sharding: data-parallel over batch, 2 batch elements per core x 8 cores.

Per-core layout: tokens on partitions (128-token subtiles), d on free dim.
- window conv = band-matrix matmul (host-built [144,128] Toeplitz bands)
- LN: bn_stats/bn_aggr per token (partition), tensor_scalar apply; LN's
  gamma/beta folded into W1/b1 on host
- FFN mm1: PE-transpose z, then W1 chunks as stationary operand, N=512
- FFN mm2: h chunks as stationary operand, W2 as moving -> output lands
  back in [token, d] layout; b2 added via a K=1 ones-outer-product matmul
- all matmuls in float32r (full-rate, ~1.5e-4 rel err); residual stream fp32
"""
import sys

sys.path.insert(0, "/opt/trn_rl_repo")

from contextlib import ExitStack

import numpy as np

import concourse.bass as bass
import concourse.tile as tile
from concourse import bacc, mybir
from concourse.bass_utils import run_bass_kernel_spmd
from concourse.masks import make_identity

B, S, D, W, L = 16, 2048, 256, 8, 3
NCORES = 8
BPC = B // NCORES            # batches per core
SUB = 128                    # tokens per subtile (partition dim)
SPT = 4                      # subtiles per supertile
ST = SUB * SPT               # tokens per supertile
NST = BPC * S // ST          # supertiles per core
F32 = mybir.dt.float32
F32R = mybir.dt.float32r
EPS = 1e-6

_CACHE = {}


def _build_program():
    nc = bacc.Bacc("TRN2", target_bir_lowering=False, debug=False, num_devices=NCORES)

    xpad_d = nc.dram_tensor("xpad", [BPC, S + 2 * W, D], F32, kind="ExternalInput").ap()
    # w1: [128(din-in-chunk), L*2br*2k*2m*128(dout-in-chunk)]
    w1_d = nc.dram_tensor("w1", [128, L * 2 * 2 * 2 * 128], F32, kind="ExternalInput").ap()
    # w2: [128(dout-in-chunk), L*2br*2m*256(d2)]
    w2_d = nc.dram_tensor("w2", [128, L * 2 * 2 * 256], F32, kind="ExternalInput").ap()
    b1_d = nc.dram_tensor("b1", [128, L * 2 * 2], F32, kind="ExternalInput").ap()
    b2_d = nc.dram_tensor("b2", [1, L * 2 * 256], F32, kind="ExternalInput").ap()
    banda_d = nc.dram_tensor("banda", [128, 2 * 128], F32, kind="ExternalInput").ap()
    bandb_d = nc.dram_tensor("bandb", [2 * W, 2 * 128], F32, kind="ExternalInput").ap()
    out_d = nc.dram_tensor("out_all", [L, BPC, S, 2 * D], F32, kind="ExternalOutput").ap()

    with tile.TileContext(nc) as tc, ExitStack() as ctx:
        const = ctx.enter_context(tc.tile_pool(name="const", bufs=1))
        stage = ctx.enter_context(tc.tile_pool(name="stage", bufs=1))
        pads = ctx.enter_context(tc.tile_pool(name="pads", bufs=3))
        xs = ctx.enter_context(tc.tile_pool(name="xs", bufs=16))
        zs = ctx.enter_context(tc.tile_pool(name="zs", bufs=8))
        zts = ctx.enter_context(tc.tile_pool(name="zts", bufs=2))
        hss = ctx.enter_context(tc.tile_pool(name="hss", bufs=2))
        stat = ctx.enter_context(tc.tile_pool(name="stat", bufs=8))
        ps_a = ctx.enter_context(tc.tile_pool(name="ps_a", bufs=2, space="PSUM"))
        ps_h = ctx.enter_context(tc.tile_pool(name="ps_h", bufs=1, space="PSUM"))
        ps_y = ctx.enter_context(tc.tile_pool(name="ps_y", bufs=3, space="PSUM"))

        # ---- constants: load fp32, round to f32r where matmuls consume them
        w1_f = stage.tile([128, L * 8 * 128], F32, tag="stg_w1")
        nc.sync.dma_start(w1_f[:], w1_d[:])
        w1_sb = const.tile([128, L, 2, 2, 2, 128], F32R)
        nc.vector.tensor_copy(w1_sb[:], w1_f[:].rearrange("p (l b k m q) -> p l b k m q", l=L, b=2, k=2, m=2))

        w2_f = stage.tile([128, L * 4 * 256], F32, tag="stg_w2")
        nc.sync.dma_start(w2_f[:], w2_d[:])
        w2_sb = const.tile([128, L, 2, 2, 256], F32R)
        nc.vector.tensor_copy(w2_sb[:], w2_f[:].rearrange("p (l b m q) -> p l b m q", l=L, b=2, m=2))

        b1_sb = const.tile([128, L, 2, 2], F32)
        nc.sync.dma_start(b1_sb[:], b1_d[:].rearrange("p (l b m) -> p l b m", l=L, b=2))

        b2_f = stage.tile([1, L * 2 * 256], F32, tag="stg_b2")
        nc.sync.dma_start(b2_f[:], b2_d[:])
        b2_sb = const.tile([1, L, 2, 256], F32R)
        nc.vector.tensor_copy(b2_sb[:], b2_f[:].rearrange("p (l b q) -> p l b q", l=L, b=2))

        banda_f = stage.tile([128, 2 * 128], F32, tag="stg_ba")
        nc.sync.dma_start(banda_f[:], banda_d[:])
        banda_sb = const.tile([128, 2, 128], F32R)
        nc.vector.tensor_copy(banda_sb[:], banda_f[:].rearrange("p (b q) -> p b q", b=2))

        bandb_f = stage.tile([2 * W, 2 * 128], F32, tag="stg_bb")
        nc.sync.dma_start(bandb_f[:], bandb_d[:])
        bandb_sb = const.tile([2 * W, 2, 128], F32R)
        nc.vector.tensor_copy(bandb_sb[:], bandb_f[:].rearrange("p (b q) -> p b q", b=2))

        ones_f = stage.tile([1, 128], F32, tag="stg_on")
        nc.vector.memset(ones_f[:], 1.0)
        ones_sb = const.tile([1, 128], F32R)
        nc.vector.tensor_copy(ones_sb[:], ones_f[:])

        ident_f = stage.tile([128, 128], F32, tag="stg_id")
        make_identity(nc, ident_f[:])
        ident = const.tile([128, 128], F32)
        nc.vector.tensor_copy(ident[:], ident_f[:])

        eps_t = const.tile([128, 1], F32)
        nc.vector.memset(eps_t[:], EPS)

        # ---- main loop
        for st in range(NST):
            b = (st * ST) // S
            t0 = (st * ST) % S

            # conv: x0[t, (br,d)] for 4 subtiles
            x_cur = []
            for sub in range(SPT):
                ts = t0 + sub * SUB
                pad_a = pads.tile([128, D], F32, tag="pad_a")
                nc.sync.dma_start(pad_a[:], xpad_d[b, ts : ts + 128, :])
                pad_b = pads.tile([2 * W, D], F32, tag="pad_b")
                nc.sync.dma_start(pad_b[:], xpad_d[b, ts + 128 : ts + 128 + 2 * W, :])
                pad_a_r = pads.tile([128, D], F32R, tag="pad_a_r")
                nc.gpsimd.tensor_copy(pad_a_r[:], pad_a[:])
                pad_b_r = pads.tile([2 * W, D], F32R, tag="pad_b_r")
                nc.gpsimd.tensor_copy(pad_b_r[:], pad_b[:])

                cps = ps_a.tile([128, 2 * D], F32, tag="ps_a")
                for br in range(2):
                    nc.tensor.matmul(
                        cps[:, br * D : (br + 1) * D],
                        banda_sb[:, br, :], pad_a_r[:], start=True, stop=False,
                    )
                    nc.tensor.matmul(
                        cps[:, br * D : (br + 1) * D],
                        bandb_sb[:, br, :], pad_b_r[:], start=False, stop=True,
                    )
                x_t = xs.tile([128, 2, D], F32, tag="x")
                nc.scalar.copy(x_t[:], cps[:].rearrange("p (b q) -> p b q", b=2))
                x_cur.append(x_t)

            for l in range(L):
                # LN stats + apply -> z (fp32)
                z_all = []
                for sub in range(SPT):
                    x_t = x_cur[sub]
                    stats = stat.tile([128, 2, 6], F32, tag="bnst")
                    mv = stat.tile([128, 2, 2], F32, tag="mv")
                    std = stat.tile([128, 2], F32, tag="std")
                    rstd = stat.tile([128, 2], F32, tag="rstd")
                    z_t = zs.tile([128, 2, D], F32, tag="z")
                    for br in range(2):
                        nc.vector.bn_stats(stats[:, br, :], x_t[:, br, :])
                        nc.vector.bn_aggr(mv[:, br, :], stats[:, br, :])
                        nc.scalar.activation(
                            std[:, br : br + 1], mv[:, br, 1:2],
                            mybir.ActivationFunctionType.Sqrt,
                            bias=eps_t[:], scale=1.0,
                        )
                        nc.vector.reciprocal(rstd[:, br : br + 1], std[:, br : br + 1])
                        nc.vector.tensor_scalar(
                            out=z_t[:, br, :], in0=x_t[:, br, :],
                            scalar1=mv[:, br, 0:1], scalar2=rstd[:, br : br + 1],
                            op0=mybir.AluOpType.subtract, op1=mybir.AluOpType.mult,
                        )
                    z_all.append(z_t)

                # transpose z -> zT (f32r via ACT copy), [din-part, (br,k,tok)]
                zt_sb = zts.tile([128, 2, 2, ST], F32R, tag="zt")
                for br in range(2):
                    for k in range(2):
                        zt_ps = ps_a.tile([128, ST], F32, tag="ps_a")
                        for sub in range(SPT):
                            nc.tensor.transpose(
                                zt_ps[:, sub * SUB : (sub + 1) * SUB],
                                z_all[sub][:, br, k * 128 : (k + 1) * 128],
                                ident[:],
                            )
                        nc.scalar.copy(zt_sb[:, br, k, :], zt_ps[:])

                # mm1 + relu -> hs (f32r), [dout-part, (br,m,tok)]
                hs_sb = hss.tile([128, 2, 2, ST], F32R, tag="hs")
                for br in range(2):
                    h_ps = ps_h.tile([128, 2, ST], F32, tag="h_ps")
                    for m in range(2):
                        for k in range(2):
                            nc.tensor.matmul(
                                h_ps[:, m, :],
                                w1_sb[:, l, br, k, m, :],
                                zt_sb[:, br, k, :],
                                start=(k == 0), stop=(k == 1),
                            )
                        nc.scalar.activation(
                            hs_sb[:, br, m, :], h_ps[:, m, :],
                            mybir.ActivationFunctionType.Relu,
                            bias=b1_sb[:, l, br, m : m + 1], scale=1.0,
                        )

                # mm2 (+b2) -> y psum [tok, (br,d)]; resid add -> x_new
                x_new_list = []
                for sub in range(SPT):
                    y_ps = ps_y.tile([128, 2, D], F32, tag="y_ps")
                    for br in range(2):
                        for m in range(2):
                            nc.tensor.matmul(
                                y_ps[:, br, :],
                                hs_sb[:, br, m, sub * SUB : (sub + 1) * SUB],
                                w2_sb[:, l, br, m, :],
                                start=(m == 0), stop=False,
                            )
                        nc.tensor.matmul(
                            y_ps[:, br, :],
                            ones_sb[:],
                            b2_sb[:, l, br, :],
                            start=False, stop=True,
                        )
                    x_new = xs.tile([128, 2, D], F32, tag="x")
                    nc.vector.tensor_tensor(
                        out=x_new[:], in0=x_cur[sub][:], in1=y_ps[:],
                        op=mybir.AluOpType.add,
                    )
                    x_new_list.append(x_new)
                    ts = t0 + sub * SUB
                    nc.sync.dma_start(
                        out_d[l, b, ts : ts + SUB, :],
                        x_new[:].rearrange("p b q -> p (b q)"),
                    )
                x_cur = x_new_list

    nc.compile()
    return nc


def _host_prep(inputs):
    x = np.asarray(inputs["inputs"], np.float32)
    lp = np.asarray(inputs["left_padding"], np.float32)
    rp = np.asarray(inputs["right_padding"], np.float32)
    lw = np.asarray(inputs["left_weights"], np.float32)
    rw = np.asarray(inputs["right_weights"], np.float32)

    xpad = np.concatenate(
        [np.broadcast_to(lp, (B, W, D)), x, np.broadcast_to(rp, (B, W, D))], axis=1
    )  # [B, S+2W, D]

    # band matrices [S+?]: out_left[t] = sum_j lw[j] pad[t+j]
    # rows r = t_in offset in [0,144), cols i = out token in [0,128)
    band = np.zeros((128 + 2 * W, 2, 128), np.float32)
    for i in range(128):
        band[i : i + W + 1, 0, i] = lw
        band[i + W : i + 2 * W + 1, 1, i] = rw
    banda = band[:128].reshape(128, 2 * 128)
    bandb = band[128:].reshape(2 * W, 2 * 128)

    # fold LN gamma/beta into W1/b1
    w1 = np.empty((L, 2, D, D), np.float32)
    b1 = np.empty((L, 2, D), np.float32)
    for bi, p in enumerate("lr"):
        pre = "left_" if p == "l" else "right_"
        g = np.asarray(inputs[pre + "ln_g"], np.float32)
        bb = np.asarray(inputs[pre + "ln_b"], np.float32)
        W1 = np.asarray(inputs[pre + "w1"], np.float32)
        B1 = np.asarray(inputs[pre + "b1"], np.float32)
        for li in range(L):
            w1[li, bi] = g[li][:, None] * W1[li]
            b1[li, bi] = B1[li] + bb[li] @ W1[li]
    w2 = np.stack(
        [np.asarray(inputs["left_w2"], np.float32), np.asarray(inputs["right_w2"], np.float32)], axis=1
    )  # [L, 2, D, D]
    b2 = np.stack(
        [np.asarray(inputs["left_b2"], np.float32), np.asarray(inputs["right_b2"], np.float32)], axis=1
    )  # [L, 2, D]

    # chunked layouts for DMA
    # w1: [din(2k x 128), dout(2m x 128)] -> [128, (l br k m q)]
    w1c = w1.reshape(L, 2, 2, 128, 2, 128).transpose(3, 0, 1, 2, 4, 5).reshape(128, -1)
    # w2: [dout(2m x 128), d2(256)] -> [128, (l br m q)]
    w2c = w2.reshape(L, 2, 2, 128, 256).transpose(3, 0, 1, 2, 4).reshape(128, -1)
    b1c = b1.reshape(L, 2, 2, 128).transpose(3, 0, 1, 2).reshape(128, -1)
    b2c = b2.reshape(1, -1)

    shared = {
        "w1": np.ascontiguousarray(w1c),
        "w2": np.ascontiguousarray(w2c),
        "b1": np.ascontiguousarray(b1c),
        "b2": np.ascontiguousarray(b2c),
        "banda": np.ascontiguousarray(banda),
        "bandb": np.ascontiguousarray(bandb),
    }
    return xpad, shared


def kernel(**inputs):
    if "nc" not in _CACHE:
        _CACHE["nc"] = _build_program()
    nc = _CACHE["nc"]

    xpad, shared = _host_prep(inputs)
    in_maps = [
        {"xpad": np.ascontiguousarray(xpad[c * BPC : (c + 1) * BPC]), **shared}
        for c in range(NCORES)
    ]
    res = run_bass_kernel_spmd(nc, in_maps, core_ids=list(range(NCORES)))

    all_layers = np.empty((L, B, S, 2 * D), np.float32)
    for c in range(NCORES):
        all_layers[:, c * BPC : (c + 1) * BPC] = res.results[c]["out_all"]
    return all_layers, all_layers[-1].copy()
